# revision 42
# baseline (speedup 1.0000x reference)
"""Trainium2 Bass kernel for CellPathwayPoolingAggregator (segment mean).

out[b, p] = (1/segment_sizes[p]) * sum_{k: segment_ids[k]==p} x[b, flat_indices[k]]

Strategy (8 cores = 2 pathway ranges x 4 batch shards):
  - Host: split the 1000 pathways into 2 contiguous ranges (<=512 pathways,
    4 pathway-tiles of <=128 each) balancing unique-gene counts. Per range,
    dedupe genes and sort them by pathway-tile signature so each K-tile of
    128 genes touches few pathway-tiles. Pack each core's working set as a
    dense DRAM tensor xtp (128, T, 512) fp16 = its range's deduped gene rows
    restricted to its batch quarter. All indexing happens on host; the
    device does plain dense HWDGE loads (8KB per-partition descriptors).
  - The two ranges share one uniform MM pattern (per-tile union of both
    ranges' pathway-tile lists) so the single SPMD program fits all cores;
    a core's S blocks are zero where its range doesn't touch the tile.
  - Device (per core): chunked dense loads (8 K-tiles = 1MB per DMA) feed
    PE matmuls with per-block count matrices S (128 genes x 128 pathways,
    stationary, fp16) accumulating into 4 PSUM banks (one per pathway-tile,
    128 pathways x 512 batch). S slices are interleaved with data chunks on
    the same HWDGE ring so the first matmul starts early. A few warm-up
    matmuls on a zeroed tile flip the PE HAM clock gate to full rate before
    real work arrives.
  - DVE/ACT scale pathway rows by 1/segment_sizes as each bank's last
    matmul retires (signature sort staggers bank completions), DMA stores
    (128, 512) f32 slices; host reassembles/transposes.
"""

import sys
from collections import Counter

import ml_dtypes
import numpy as np

_TRN_REPO = "/opt/trn_rl_repo"
if _TRN_REPO not in sys.path:
    sys.path.insert(0, _TRN_REPO)

import concourse.bass as bass  # noqa: F401
import concourse.mybir as mybir
import concourse.tile as tile
from concourse import bacc
from concourse.bass_utils import run_bass_kernel_spmd

B, G, P = 2048, 10000, 1000
NCORES = 8
NPT = 4           # pathway tiles per range
BQ = 512          # batch columns per core (B / 4 shards)
CH_MAX = 8        # K-tiles per dense load chunk (8KB/partition descriptors)
N_WARM = 13       # PE warm-up matmuls: bridge the pipe-FIFO ramp so HAM
                  # never re-throttles before steady state
N_FP8_CHUNKS = 3  # leading chunks whose S rides as fp8: halves the ramp
                  # bytes that gate early matmuls; their mixed-dtype matmul
                  # slowdown (259 vs 215ns) lands in PE-idle ramp time


def _chunk_list(T):
    """Small first chunk (fast first MM), CH_MAX middles, small final
    chunks so the PE trail after the last load is short. (A graduated
    2,3,4,6 ramp was measured 4us SLOWER — per-DMA overheads dominate.)"""
    if T <= 2:
        return [T]
    chunks = [2]
    rem = T - 2
    while rem > CH_MAX + 3:
        chunks.append(CH_MAX)
        rem -= CH_MAX
    if rem > 3:
        chunks.append(rem - 3)
        rem = 3
    if rem:
        chunks.append(rem)
    return chunks


def _build_schedule(flat_indices, segment_ids):
    seg = np.asarray(segment_ids, dtype=np.int64)
    idx = np.asarray(flat_indices, dtype=np.int64)
    order = np.argsort(seg, kind="stable")
    seg, idx = seg[order], idx[order]
    seg_starts = np.searchsorted(seg, np.arange(P + 1))

    # range boundary balancing unique-gene counts (range sizes <= NPT*128)
    best, best_cost = None, None
    for b in range(P - NPT * 128, NPT * 128 + 1):
        uA = len(np.unique(idx[: seg_starts[b]]))
        uB = len(np.unique(idx[seg_starts[b] :]))
        cost = max(uA, uB)
        if best_cost is None or cost < best_cost:
            best, best_cost = b, cost
    bounds = [0, best, P]

    ranges = []
    for R in range(2):
        lo_p, hi_p = bounds[R], bounds[R + 1]
        lo, hi = seg_starts[lo_p], seg_starts[hi_p]
        genes = idx[lo:hi]
        lseg = seg[lo:hi] - lo_p
        pt = lseg // 128
        sig = {}
        for g, p_ in zip(genes.tolist(), pt.tolist()):
            sig.setdefault(g, set()).add(p_)
        genes_sorted = sorted(sig.keys(), key=lambda g: (tuple(sorted(sig[g])), g))
        ranges.append((lo_p, hi_p, genes, lseg, sig, genes_sorted))

    T2 = max((len(r[5]) + 127) // 128 for r in ranges)
    Kpad = T2 * 128

    tile_pts, gene_pads = [], []
    for lo_p, hi_p, genes, lseg, sig, gs in ranges:
        gpad = gs + [-1] * (Kpad - len(gs))
        gene_pads.append(gpad)
        L = []
        for t in range(T2):
            un = set()
            for g in gpad[t * 128 : (t + 1) * 128]:
                if g >= 0:
                    un.update(sig[g])
            L.append(sorted(un))
        tile_pts.append(L)

    pattern = [sorted(set(tile_pts[0][t]) | set(tile_pts[1][t])) for t in range(T2)]
    blocks = [(t, p_) for t in range(T2) for p_ in pattern[t]]
    M = len(blocks)
    first_touch, last_touch = {}, {}
    for m, (t, p_) in enumerate(blocks):
        first_touch.setdefault(p_, m)
        last_touch[p_] = m

    block_of = {tp: m for m, tp in enumerate(blocks)}
    smats = []
    for R, (lo_p, hi_p, genes, lseg, sig, gs) in enumerate(ranges):
        gpad = gene_pads[R]
        pos = {g: j for j, g in enumerate(gpad) if g >= 0}
        S = np.zeros((128, M * 128), np.float32)
        cnt = Counter(zip(genes.tolist(), lseg.tolist()))
        for (g, lp), c in cnt.items():
            j = pos[g]
            m = block_of[(j // 128, lp // 128)]
            S[j % 128, m * 128 + (lp % 128)] += c
        # kept f32 here; _prepare casts per chunk (fp8 ramp / fp16 steady)
        smats.append(S)

    chunks = _chunk_list(T2)
    mranges, t0 = [], 0
    for ch in chunks:
        m0 = sum(len(pattern[t]) for t in range(t0))
        m1 = m0 + sum(len(pattern[t]) for t in range(t0, t0 + ch))
        mranges.append((m0, m1))
        t0 += ch

    return dict(
        bounds=bounds, T2=T2, blocks=blocks,
        first_touch=first_touch, last_touch=last_touch,
        gene_pads=gene_pads, smats=smats, chunks=chunks, mranges=mranges,
    )


def _build_program(sch):
    nc = bacc.Bacc(
        "TRN2",
        target_bir_lowering=False,
        debug=False,
        num_devices=NCORES,
    )
    f16, f32, f8 = mybir.dt.float16, mybir.dt.float32, mybir.dt.float8e4

    T2 = sch["T2"]
    blocks = sch["blocks"]
    chunks = sch["chunks"]
    mranges = sch["mranges"]
    first_touch, last_touch = sch["first_touch"], sch["last_touch"]

    # Ramp chunks keep separate fp8-S + data tensors; steady chunks merge
    # x and S into ONE DRAM tensor / ONE DMA each (fewer desc issues, sem
    # lanes, and chunk-boundary waits — 20 loads measured 13.3us of Sync
    # issue time + ~60 sem ops per engine).
    t_ramp = sum(chunks[:N_FP8_CHUNKS])
    xtp_d = nc.dram_tensor("xtp", [128, t_ramp, BQ], f16, kind="ExternalInput")
    s_ds, c_ds = [], []
    for ci, (m0, m1) in enumerate(mranges):
        ch = chunks[ci]
        if ci < N_FP8_CHUNKS:
            s_ds.append(
                nc.dram_tensor(
                    f"s{ci}", [128, (m1 - m0) * 128], f8, kind="ExternalInput"
                )
            )
            c_ds.append(None)
        else:
            s_ds.append(None)
            c_ds.append(
                nc.dram_tensor(
                    f"c{ci}",
                    [128, ch * BQ + (m1 - m0) * 128],
                    f16,
                    kind="ExternalInput",
                )
            )
    inv_d = nc.dram_tensor("invsz", [128, NPT], f32, kind="ExternalInput")
    out_d = nc.dram_tensor("out", [NPT * 128, BQ], f16, kind="ExternalOutput")

    with tile.TileContext(nc) as tc:
        with (
            tc.tile_pool(name="const", bufs=1) as cpool,
            tc.tile_pool(name="warmp", bufs=1) as wpool,
            tc.tile_pool(name="psum", bufs=1, space="PSUM") as ppool,
            tc.tile_pool(name="outp", bufs=1) as opool,
        ):
            # PE warm-up: zeroed operands, separate PSUM bank. Runs while the
            # first S/x chunks stream in, flipping HAM to 8/8 early.
            warm_sb = wpool.tile([128, 640], f16, tag="warm")
            nc.gpsimd.memset(warm_sb[:], 0.0)
            warm_ps = ppool.tile([128, 512], f32, tag="wps", name="wps")
            for i in range(N_WARM):
                nc.tensor.matmul(
                    warm_ps[:],
                    warm_sb[:, 512:640],
                    warm_sb[:, 0:512],
                    start=(i == 0),
                    stop=(i == N_WARM - 1),
                )
            warm_out = wpool.tile([128, 512], f32, tag="warmo")
            nc.vector.tensor_copy(warm_out[:], warm_ps[:])

            inv_sb = cpool.tile([128, NPT], f32, tag="invsz")
            nc.scalar.dma_start(inv_sb[:], inv_d.ap())

            psb = [
                ppool.tile([128, 512], f32, tag=f"ps{n}", name=f"ps{n}")
                for n in range(NPT)
            ]
            s_sbs = []

            # Evictions (DVE) run as each bank's last matmul retires;
            # stores happen at the end via SWDGE (see below).
            ots = {}

            def evict(pt):
                ot = opool.tile([128, 512], f16, tag=f"ot{pt}", name=f"ot{pt}")
                nc.vector.tensor_scalar_mul(
                    ot[:], psb[pt][:], inv_sb[:, pt : pt + 1]
                )
                ots[pt] = ot

            t0 = 0
            last_load = None
            for ci, ch in enumerate(chunks):
                m0, m1 = mranges[ci]
                # All loads ride the Sync HWDGE ring in arrival order. (The
                # Scalar/ACT ring starves whenever the Sync ring has backlog
                # — observed 6us S-arrival delays.)
                if ci < N_FP8_CHUNKS:
                    s_sb = cpool.tile(
                        [128, (m1 - m0) * 128], f8, tag=f"s{ci}", name=f"s{ci}"
                    )
                    nc.sync.dma_start(s_sb[:], s_ds[ci].ap())
                    gt = cpool.tile(
                        [128, ch, BQ], f16, tag=f"gt{ci}", name=f"gt{ci}"
                    )
                    last_load = nc.sync.dma_start(
                        gt[:], xtp_d.ap()[:, t0 : t0 + ch, :]
                    )

                    def rhs_ap(tl, _gt=gt):
                        return _gt[:, tl, :]

                    def lhs_ap(mi, _s=s_sb):
                        return _s[:, mi * 128 : (mi + 1) * 128]
                else:
                    ct = cpool.tile(
                        [128, ch * BQ + (m1 - m0) * 128],
                        f16,
                        tag=f"c{ci}",
                        name=f"c{ci}",
                    )
                    last_load = nc.sync.dma_start(ct[:], c_ds[ci].ap())

                    def rhs_ap(tl, _ct=ct):
                        return _ct[:, tl * BQ : (tl + 1) * BQ]

                    def lhs_ap(mi, _ct=ct, _off=ch * BQ):
                        return _ct[:, _off + mi * 128 : _off + (mi + 1) * 128]

                for m in range(m0, m1):
                    tt, pt = blocks[m]
                    tl = tt - t0
                    nc.tensor.matmul(
                        psb[pt][:],
                        lhs_ap(m - m0),
                        rhs_ap(tl),
                        start=(m == first_touch[pt]),
                        stop=(m == last_touch[pt]),
                    )
                    if m == last_touch[pt]:
                        evict(pt)
                t0 += ch

            # Stores on the Scalar HWDGE ring, explicitly ordered after the
            # last load: the 8 DMAHW sem lanes are recycled in scheduled
            # order, and a store scheduled mid-stream makes a later LOAD
            # wait on store->eviction->matmul (observed 8us stall). The
            # forced dep keeps every store behind every load in lane order.
            for pt in range(NPT):
                st = nc.scalar.dma_start(
                    out_d.ap()[pt * 128 : (pt + 1) * 128, :], ots[pt][:]
                )
                bass._add_dep_helper(
                    st.ins, last_load.ins, sync=True,
                    reason="stores after all loads (DMAHW lane ordering)",
                )
    return nc


def _prepare(gene_set_features, flat_indices, segment_ids, segment_sizes):
    sch = _build_schedule(flat_indices, segment_ids)
    nc = _build_program(sch)
    nc.compile()

    x = np.asarray(gene_set_features, dtype=np.float32)
    xt16 = np.ascontiguousarray(x.T.astype(np.float16))  # (G, B)
    sizes = np.asarray(segment_sizes, dtype=np.float32)

    T2 = sch["T2"]
    bounds = sch["bounds"]
    # per-range shared arrays
    s_pieces, invs, gpads = [], [], []
    for R in range(2):
        lo_p, hi_p = bounds[R], bounds[R + 1]
        S = sch["smats"][R]
        s_pieces.append(
            [
                np.ascontiguousarray(
                    S[:, m0 * 128 : m1 * 128].astype(
                        ml_dtypes.float8_e4m3 if ci < N_FP8_CHUNKS else np.float16
                    )
                )
                for ci, (m0, m1) in enumerate(sch["mranges"])
            ]
        )
        inv = np.ones((128, NPT), np.float32)
        for pt in range(NPT):
            lo_row = lo_p + pt * 128
            n = min(128, hi_p - lo_row)
            if n > 0:
                inv[:n, pt] = 1.0 / sizes[lo_row : lo_row + n]
        invs.append(inv)
        gpads.append(
            np.array([g if g >= 0 else 0 for g in sch["gene_pads"][R]], np.int64)
        )

    chunks = sch["chunks"]
    t_ramp = sum(chunks[:N_FP8_CHUNKS])
    in_maps = []
    for c in range(NCORES):
        R, q = c // 4, c % 4
        xtp = np.ascontiguousarray(
            xt16[gpads[R], q * BQ : (q + 1) * BQ]
            .reshape(T2, 128, BQ)
            .transpose(1, 0, 2)
        )
        im = {"xtp": np.ascontiguousarray(xtp[:, :t_ramp, :]), "invsz": invs[R]}
        t0 = 0
        for ci, ch in enumerate(chunks):
            if ci < N_FP8_CHUNKS:
                im[f"s{ci}"] = s_pieces[R][ci]
            else:
                im[f"c{ci}"] = np.ascontiguousarray(
                    np.concatenate(
                        [
                            xtp[:, t0 : t0 + ch, :].reshape(128, ch * BQ),
                            s_pieces[R][ci],
                        ],
                        axis=1,
                    )
                )
            t0 += ch
        in_maps.append(im)
    return nc, in_maps, sch


def _unshard(res, sch):
    bounds = sch["bounds"]
    out = np.empty((B, P), np.float32)
    for c in range(NCORES):
        R, q = c // 4, c % 4
        lo_p, hi_p = bounds[R], bounds[R + 1]
        o = np.asarray(res.results[c]["out"]).astype(np.float32)  # (NPT*128, BQ)
        for pt in range(NPT):
            lo_row = lo_p + pt * 128
            n = min(128, hi_p - lo_row)
            if n > 0:
                out[q * BQ : (q + 1) * BQ, lo_row : lo_row + n] = o[
                    pt * 128 : pt * 128 + n
                ].T
    return np.ascontiguousarray(out)


def kernel(gene_set_features, flat_indices, segment_ids, segment_sizes, _res_hook=None):
    nc, in_maps, sch = _prepare(
        gene_set_features, flat_indices, segment_ids, segment_sizes
    )
    res = run_bass_kernel_spmd(nc, in_maps, list(range(NCORES)))
    if _res_hook is not None:
        _res_hook(res)
    return _unshard(res, sch)


# revision 44
# speedup vs baseline: 1.0400x; 1.0400x over previous
"""Trainium2 Bass kernel for CellPathwayPoolingAggregator (segment mean).

out[b, p] = (1/segment_sizes[p]) * sum_{k: segment_ids[k]==p} x[b, flat_indices[k]]

Strategy (8 cores = 2 pathway ranges x 4 batch shards):
  - Host: split the 1000 pathways into 2 contiguous ranges (<=512 pathways,
    4 pathway-tiles of <=128 each) balancing unique-gene counts. Per range,
    dedupe genes and sort them by pathway-tile signature so each K-tile of
    128 genes touches few pathway-tiles. Pack each core's working set as a
    dense DRAM tensor xtp (128, T, 512) fp16 = its range's deduped gene rows
    restricted to its batch quarter. All indexing happens on host; the
    device does plain dense HWDGE loads (8KB per-partition descriptors).
  - The two ranges share one uniform MM pattern (per-tile union of both
    ranges' pathway-tile lists) so the single SPMD program fits all cores;
    a core's S blocks are zero where its range doesn't touch the tile.
  - Device (per core): chunked dense loads (8 K-tiles = 1MB per DMA) feed
    PE matmuls with per-block count matrices S (128 genes x 128 pathways,
    stationary, fp16) accumulating into 4 PSUM banks (one per pathway-tile,
    128 pathways x 512 batch). S slices are interleaved with data chunks on
    the same HWDGE ring so the first matmul starts early. A few warm-up
    matmuls on a zeroed tile flip the PE HAM clock gate to full rate before
    real work arrives.
  - DVE/ACT scale pathway rows by 1/segment_sizes as each bank's last
    matmul retires (signature sort staggers bank completions), DMA stores
    (128, 512) f32 slices; host reassembles/transposes.
"""

import sys
from collections import Counter

import ml_dtypes
import numpy as np

_TRN_REPO = "/opt/trn_rl_repo"
if _TRN_REPO not in sys.path:
    sys.path.insert(0, _TRN_REPO)

import concourse.bass as bass  # noqa: F401
import concourse.mybir as mybir
import concourse.tile as tile
from concourse import bacc
from concourse.bass_utils import run_bass_kernel_spmd

B, G, P = 2048, 10000, 1000
NCORES = 8
NPT = 4           # pathway tiles per range
BQ = 512          # batch columns per core (B / 4 shards)
CH_MAX = 8        # K-tiles per dense load chunk (8KB/partition descriptors)
N_WARM = 7        # PE warm-up matmuls: with the descending-signature sort
                  # real ramp work arrives early, so only a short HAM
                  # warm-up bridge is needed
N_FP8_CHUNKS = 3  # leading chunks whose S rides as fp8: halves the ramp
                  # bytes that gate early matmuls; their mixed-dtype matmul
                  # slowdown (259 vs 215ns) lands in PE-idle ramp time


def _chunk_list(T):
    """Small first chunk (fast first MM), CH_MAX middles, small final
    chunks so the PE trail after the last load is short. (A graduated
    2,3,4,6 ramp was measured 4us SLOWER — per-DMA overheads dominate.)"""
    if T <= 2:
        return [T]
    chunks = [2]
    rem = T - 2
    while rem > CH_MAX + 3:
        chunks.append(CH_MAX)
        rem -= CH_MAX
    if rem > 3:
        chunks.append(rem - 3)
        rem = 3
    if rem:
        chunks.append(rem)
    return chunks


def _build_schedule(flat_indices, segment_ids):
    seg = np.asarray(segment_ids, dtype=np.int64)
    idx = np.asarray(flat_indices, dtype=np.int64)
    order = np.argsort(seg, kind="stable")
    seg, idx = seg[order], idx[order]
    seg_starts = np.searchsorted(seg, np.arange(P + 1))

    # range boundary balancing unique-gene counts (range sizes <= NPT*128)
    best, best_cost = None, None
    for b in range(P - NPT * 128, NPT * 128 + 1):
        uA = len(np.unique(idx[: seg_starts[b]]))
        uB = len(np.unique(idx[seg_starts[b] :]))
        cost = max(uA, uB)
        if best_cost is None or cost < best_cost:
            best, best_cost = b, cost
    bounds = [0, best, P]

    ranges = []
    for R in range(2):
        lo_p, hi_p = bounds[R], bounds[R + 1]
        lo, hi = seg_starts[lo_p], seg_starts[hi_p]
        genes = idx[lo:hi]
        lseg = seg[lo:hi] - lo_p
        pt = lseg // 128
        sig = {}
        for g, p_ in zip(genes.tolist(), pt.tolist()):
            sig.setdefault(g, set()).add(p_)
        # Descending signature size: multi-pathway-tile genes first. Their
        # tiles carry 3-4 matmul blocks each, so the ramp chunks (whose
        # arrival the PE would otherwise idle-wait on) deliver ~2.5x more
        # PE work per byte; the single-tile genes stream late when the
        # pipe is far ahead of the PE.
        genes_sorted = sorted(
            sig.keys(), key=lambda g: (-len(sig[g]), tuple(sorted(sig[g])), g)
        )
        ranges.append((lo_p, hi_p, genes, lseg, sig, genes_sorted))

    T2 = max((len(r[5]) + 127) // 128 for r in ranges)
    Kpad = T2 * 128

    tile_pts, gene_pads = [], []
    for lo_p, hi_p, genes, lseg, sig, gs in ranges:
        gpad = gs + [-1] * (Kpad - len(gs))
        gene_pads.append(gpad)
        L = []
        for t in range(T2):
            un = set()
            for g in gpad[t * 128 : (t + 1) * 128]:
                if g >= 0:
                    un.update(sig[g])
            L.append(sorted(un))
        tile_pts.append(L)

    pattern = [sorted(set(tile_pts[0][t]) | set(tile_pts[1][t])) for t in range(T2)]
    blocks = [(t, p_) for t in range(T2) for p_ in pattern[t]]
    M = len(blocks)
    first_touch, last_touch = {}, {}
    for m, (t, p_) in enumerate(blocks):
        first_touch.setdefault(p_, m)
        last_touch[p_] = m

    block_of = {tp: m for m, tp in enumerate(blocks)}
    smats = []
    for R, (lo_p, hi_p, genes, lseg, sig, gs) in enumerate(ranges):
        gpad = gene_pads[R]
        pos = {g: j for j, g in enumerate(gpad) if g >= 0}
        S = np.zeros((128, M * 128), np.float32)
        cnt = Counter(zip(genes.tolist(), lseg.tolist()))
        for (g, lp), c in cnt.items():
            j = pos[g]
            m = block_of[(j // 128, lp // 128)]
            S[j % 128, m * 128 + (lp % 128)] += c
        # kept f32 here; _prepare casts per chunk (fp8 ramp / fp16 steady)
        smats.append(S)

    chunks = _chunk_list(T2)
    mranges, t0 = [], 0
    for ch in chunks:
        m0 = sum(len(pattern[t]) for t in range(t0))
        m1 = m0 + sum(len(pattern[t]) for t in range(t0, t0 + ch))
        mranges.append((m0, m1))
        t0 += ch

    return dict(
        bounds=bounds, T2=T2, blocks=blocks,
        first_touch=first_touch, last_touch=last_touch,
        gene_pads=gene_pads, smats=smats, chunks=chunks, mranges=mranges,
    )


def _build_program(sch):
    nc = bacc.Bacc(
        "TRN2",
        target_bir_lowering=False,
        debug=False,
        num_devices=NCORES,
    )
    f16, f32, f8 = mybir.dt.float16, mybir.dt.float32, mybir.dt.float8e4

    T2 = sch["T2"]
    blocks = sch["blocks"]
    chunks = sch["chunks"]
    mranges = sch["mranges"]
    first_touch, last_touch = sch["first_touch"], sch["last_touch"]

    # Ramp chunks keep separate fp8-S + data tensors; steady chunks merge
    # x and S into ONE DRAM tensor / ONE DMA each (fewer desc issues, sem
    # lanes, and chunk-boundary waits — 20 loads measured 13.3us of Sync
    # issue time + ~60 sem ops per engine).
    t_ramp = sum(chunks[:N_FP8_CHUNKS])
    xtp_d = nc.dram_tensor("xtp", [128, t_ramp, BQ], f16, kind="ExternalInput")
    s_ds, c_ds = [], []
    for ci, (m0, m1) in enumerate(mranges):
        ch = chunks[ci]
        if ci < N_FP8_CHUNKS:
            s_ds.append(
                nc.dram_tensor(
                    f"s{ci}", [128, (m1 - m0) * 128], f8, kind="ExternalInput"
                )
            )
            c_ds.append(None)
        else:
            s_ds.append(None)
            c_ds.append(
                nc.dram_tensor(
                    f"c{ci}",
                    [128, ch * BQ + (m1 - m0) * 128],
                    f16,
                    kind="ExternalInput",
                )
            )
    inv_d = nc.dram_tensor("invsz", [128, NPT], f32, kind="ExternalInput")
    out_d = nc.dram_tensor("out", [NPT * 128, BQ], f16, kind="ExternalOutput")

    with tile.TileContext(nc) as tc:
        with (
            tc.tile_pool(name="const", bufs=1) as cpool,
            tc.tile_pool(name="warmp", bufs=1) as wpool,
            tc.tile_pool(name="psum", bufs=1, space="PSUM") as ppool,
            tc.tile_pool(name="outp", bufs=1) as opool,
        ):
            # PE warm-up: zeroed operands, separate PSUM bank. Runs while the
            # first S/x chunks stream in, flipping HAM to 8/8 early.
            warm_sb = wpool.tile([128, 640], f16, tag="warm")
            nc.gpsimd.memset(warm_sb[:], 0.0)
            warm_ps = ppool.tile([128, 512], f32, tag="wps", name="wps")
            for i in range(N_WARM):
                nc.tensor.matmul(
                    warm_ps[:],
                    warm_sb[:, 512:640],
                    warm_sb[:, 0:512],
                    start=(i == 0),
                    stop=(i == N_WARM - 1),
                )
            warm_out = wpool.tile([128, 512], f32, tag="warmo")
            nc.vector.tensor_copy(warm_out[:], warm_ps[:])

            inv_sb = cpool.tile([128, NPT], f32, tag="invsz")
            nc.scalar.dma_start(inv_sb[:], inv_d.ap())

            psb = [
                ppool.tile([128, 512], f32, tag=f"ps{n}", name=f"ps{n}")
                for n in range(NPT)
            ]
            s_sbs = []

            # Evictions (DVE) run as each bank's last matmul retires;
            # stores happen at the end via SWDGE (see below).
            ots = {}

            def evict(pt):
                ot = opool.tile([128, 512], f16, tag=f"ot{pt}", name=f"ot{pt}")
                nc.vector.tensor_scalar_mul(
                    ot[:], psb[pt][:], inv_sb[:, pt : pt + 1]
                )
                ots[pt] = ot

            t0 = 0
            last_load = None
            for ci, ch in enumerate(chunks):
                m0, m1 = mranges[ci]
                # All loads ride the Sync HWDGE ring in arrival order. (The
                # Scalar/ACT ring starves whenever the Sync ring has backlog
                # — observed 6us S-arrival delays.)
                if ci < N_FP8_CHUNKS:
                    s_sb = cpool.tile(
                        [128, (m1 - m0) * 128], f8, tag=f"s{ci}", name=f"s{ci}"
                    )
                    nc.sync.dma_start(s_sb[:], s_ds[ci].ap())
                    gt = cpool.tile(
                        [128, ch, BQ], f16, tag=f"gt{ci}", name=f"gt{ci}"
                    )
                    last_load = nc.sync.dma_start(
                        gt[:], xtp_d.ap()[:, t0 : t0 + ch, :]
                    )

                    def rhs_ap(tl, _gt=gt):
                        return _gt[:, tl, :]

                    def lhs_ap(mi, _s=s_sb):
                        return _s[:, mi * 128 : (mi + 1) * 128]
                else:
                    ct = cpool.tile(
                        [128, ch * BQ + (m1 - m0) * 128],
                        f16,
                        tag=f"c{ci}",
                        name=f"c{ci}",
                    )
                    last_load = nc.sync.dma_start(ct[:], c_ds[ci].ap())

                    def rhs_ap(tl, _ct=ct):
                        return _ct[:, tl * BQ : (tl + 1) * BQ]

                    def lhs_ap(mi, _ct=ct, _off=ch * BQ):
                        return _ct[:, _off + mi * 128 : _off + (mi + 1) * 128]

                for m in range(m0, m1):
                    tt, pt = blocks[m]
                    tl = tt - t0
                    nc.tensor.matmul(
                        psb[pt][:],
                        lhs_ap(m - m0),
                        rhs_ap(tl),
                        start=(m == first_touch[pt]),
                        stop=(m == last_touch[pt]),
                    )
                    if m == last_touch[pt]:
                        evict(pt)
                t0 += ch

            # Stores on the Scalar HWDGE ring, explicitly ordered after the
            # last load: the 8 DMAHW sem lanes are recycled in scheduled
            # order, and a store scheduled mid-stream makes a later LOAD
            # wait on store->eviction->matmul (observed 8us stall). The
            # forced dep keeps every store behind every load in lane order.
            for pt in range(NPT):
                st = nc.scalar.dma_start(
                    out_d.ap()[pt * 128 : (pt + 1) * 128, :], ots[pt][:]
                )
                bass._add_dep_helper(
                    st.ins, last_load.ins, sync=True,
                    reason="stores after all loads (DMAHW lane ordering)",
                )
    return nc


def _prepare(gene_set_features, flat_indices, segment_ids, segment_sizes):
    sch = _build_schedule(flat_indices, segment_ids)
    nc = _build_program(sch)
    nc.compile()

    x = np.asarray(gene_set_features, dtype=np.float32)
    xt16 = np.ascontiguousarray(x.T.astype(np.float16))  # (G, B)
    sizes = np.asarray(segment_sizes, dtype=np.float32)

    T2 = sch["T2"]
    bounds = sch["bounds"]
    # per-range shared arrays
    s_pieces, invs, gpads = [], [], []
    for R in range(2):
        lo_p, hi_p = bounds[R], bounds[R + 1]
        S = sch["smats"][R]
        s_pieces.append(
            [
                np.ascontiguousarray(
                    S[:, m0 * 128 : m1 * 128].astype(
                        ml_dtypes.float8_e4m3 if ci < N_FP8_CHUNKS else np.float16
                    )
                )
                for ci, (m0, m1) in enumerate(sch["mranges"])
            ]
        )
        inv = np.ones((128, NPT), np.float32)
        for pt in range(NPT):
            lo_row = lo_p + pt * 128
            n = min(128, hi_p - lo_row)
            if n > 0:
                inv[:n, pt] = 1.0 / sizes[lo_row : lo_row + n]
        invs.append(inv)
        gpads.append(
            np.array([g if g >= 0 else 0 for g in sch["gene_pads"][R]], np.int64)
        )

    chunks = sch["chunks"]
    t_ramp = sum(chunks[:N_FP8_CHUNKS])
    in_maps = []
    for c in range(NCORES):
        R, q = c // 4, c % 4
        xtp = np.ascontiguousarray(
            xt16[gpads[R], q * BQ : (q + 1) * BQ]
            .reshape(T2, 128, BQ)
            .transpose(1, 0, 2)
        )
        im = {"xtp": np.ascontiguousarray(xtp[:, :t_ramp, :]), "invsz": invs[R]}
        t0 = 0
        for ci, ch in enumerate(chunks):
            if ci < N_FP8_CHUNKS:
                im[f"s{ci}"] = s_pieces[R][ci]
            else:
                im[f"c{ci}"] = np.ascontiguousarray(
                    np.concatenate(
                        [
                            xtp[:, t0 : t0 + ch, :].reshape(128, ch * BQ),
                            s_pieces[R][ci],
                        ],
                        axis=1,
                    )
                )
            t0 += ch
        in_maps.append(im)
    return nc, in_maps, sch


def _unshard(res, sch):
    bounds = sch["bounds"]
    out = np.empty((B, P), np.float32)
    for c in range(NCORES):
        R, q = c // 4, c % 4
        lo_p, hi_p = bounds[R], bounds[R + 1]
        o = np.asarray(res.results[c]["out"]).astype(np.float32)  # (NPT*128, BQ)
        for pt in range(NPT):
            lo_row = lo_p + pt * 128
            n = min(128, hi_p - lo_row)
            if n > 0:
                out[q * BQ : (q + 1) * BQ, lo_row : lo_row + n] = o[
                    pt * 128 : pt * 128 + n
                ].T
    return np.ascontiguousarray(out)


def kernel(gene_set_features, flat_indices, segment_ids, segment_sizes, _res_hook=None):
    nc, in_maps, sch = _prepare(
        gene_set_features, flat_indices, segment_ids, segment_sizes
    )
    res = run_bass_kernel_spmd(nc, in_maps, list(range(NCORES)))
    if _res_hook is not None:
        _res_hook(res)
    return _unshard(res, sch)


# revision 58
# speedup vs baseline: 1.0652x; 1.0243x over previous
"""Trainium2 Bass kernel for CellPathwayPoolingAggregator (segment mean).

out[b, p] = (1/segment_sizes[p]) * sum_{k: segment_ids[k]==p} x[b, flat_indices[k]]

Strategy (8 cores = 2 pathway ranges x 4 batch shards):
  - Host: split the 1000 pathways into 2 contiguous ranges (<=512 pathways,
    4 pathway-tiles of <=128 each) balancing unique-gene counts. Per range,
    dedupe genes and sort them by pathway-tile signature so each K-tile of
    128 genes touches few pathway-tiles. Pack each core's working set as a
    dense DRAM tensor xtp (128, T, 512) fp16 = its range's deduped gene rows
    restricted to its batch quarter. All indexing happens on host; the
    device does plain dense HWDGE loads (8KB per-partition descriptors).
  - The two ranges share one uniform MM pattern (per-tile union of both
    ranges' pathway-tile lists) so the single SPMD program fits all cores;
    a core's S blocks are zero where its range doesn't touch the tile.
  - Device (per core): chunked dense loads (8 K-tiles = 1MB per DMA) feed
    PE matmuls with per-block count matrices S (128 genes x 128 pathways,
    stationary, fp16) accumulating into 4 PSUM banks (one per pathway-tile,
    128 pathways x 512 batch). S slices are interleaved with data chunks on
    the same HWDGE ring so the first matmul starts early. A few warm-up
    matmuls on a zeroed tile flip the PE HAM clock gate to full rate before
    real work arrives.
  - DVE/ACT scale pathway rows by 1/segment_sizes as each bank's last
    matmul retires (signature sort staggers bank completions), DMA stores
    (128, 512) f32 slices; host reassembles/transposes.
"""

import sys
from collections import Counter

import ml_dtypes
import numpy as np

_TRN_REPO = "/opt/trn_rl_repo"
if _TRN_REPO not in sys.path:
    sys.path.insert(0, _TRN_REPO)

import concourse.bass as bass  # noqa: F401
import concourse.mybir as mybir
import concourse.tile as tile
from concourse import bacc
from concourse.bass_utils import run_bass_kernel_spmd

B, G, P = 2048, 10000, 1000
NCORES = 8
NPT = 4           # pathway tiles per range
BQ = 512          # batch columns per core (B / 4 shards)
CH_MAX = 8        # K-tiles per dense load chunk (8KB/partition descriptors)
N_WARM = 13       # PE warm-up matmuls: bridge the pipe-FIFO ramp so HAM
                  # never re-throttles before steady state
N_FP8_CHUNKS = 3  # leading chunks whose S rides as fp8: halves the ramp
                  # bytes that gate early matmuls; their mixed-dtype matmul
                  # slowdown (259 vs 215ns) lands in PE-idle ramp time
                  # (N_FP8=1 and all-fp16-merged both measured worse)


def _chunk_list(T):
    """Small first chunk (fast first MM), CH_MAX middles, small final
    chunks so the PE trail after the last load is short. (A graduated
    2,3,4,6 ramp was measured 4us SLOWER — per-DMA overheads dominate.)"""
    if T <= 2:
        return [T]
    chunks = [2]
    rem = T - 2
    while rem > CH_MAX + 3:
        chunks.append(CH_MAX)
        rem -= CH_MAX
    if rem > 3:
        chunks.append(rem - 3)
        rem = 3
    if rem:
        chunks.append(rem)
    return chunks


def _build_schedule(flat_indices, segment_ids):
    seg = np.asarray(segment_ids, dtype=np.int64)
    idx = np.asarray(flat_indices, dtype=np.int64)
    order = np.argsort(seg, kind="stable")
    seg, idx = seg[order], idx[order]
    seg_starts = np.searchsorted(seg, np.arange(P + 1))

    # range boundary balancing unique-gene counts (range sizes <= NPT*128)
    best, best_cost = None, None
    for b in range(P - NPT * 128, NPT * 128 + 1):
        uA = len(np.unique(idx[: seg_starts[b]]))
        uB = len(np.unique(idx[seg_starts[b] :]))
        cost = max(uA, uB)
        if best_cost is None or cost < best_cost:
            best, best_cost = b, cost
    bounds = [0, best, P]

    ranges = []
    for R in range(2):
        lo_p, hi_p = bounds[R], bounds[R + 1]
        lo, hi = seg_starts[lo_p], seg_starts[hi_p]
        genes = idx[lo:hi]
        lseg = seg[lo:hi] - lo_p
        pt = lseg // 128
        sig = {}
        for g, p_ in zip(genes.tolist(), pt.tolist()):
            sig.setdefault(g, set()).add(p_)
        genes_sorted = sorted(sig.keys(), key=lambda g: (tuple(sorted(sig[g])), g))
        ranges.append((lo_p, hi_p, genes, lseg, sig, genes_sorted))

    T2 = max((len(r[5]) + 127) // 128 for r in ranges)
    Kpad = T2 * 128

    tile_pts, gene_pads = [], []
    for lo_p, hi_p, genes, lseg, sig, gs in ranges:
        gpad = gs + [-1] * (Kpad - len(gs))
        gene_pads.append(gpad)
        L = []
        for t in range(T2):
            un = set()
            for g in gpad[t * 128 : (t + 1) * 128]:
                if g >= 0:
                    un.update(sig[g])
            L.append(sorted(un))
        tile_pts.append(L)

    pattern = [sorted(set(tile_pts[0][t]) | set(tile_pts[1][t])) for t in range(T2)]
    blocks = [(t, p_) for t in range(T2) for p_ in pattern[t]]
    M = len(blocks)
    first_touch, last_touch = {}, {}
    for m, (t, p_) in enumerate(blocks):
        first_touch.setdefault(p_, m)
        last_touch[p_] = m

    block_of = {tp: m for m, tp in enumerate(blocks)}
    smats = []
    for R, (lo_p, hi_p, genes, lseg, sig, gs) in enumerate(ranges):
        gpad = gene_pads[R]
        pos = {g: j for j, g in enumerate(gpad) if g >= 0}
        S = np.zeros((128, M * 128), np.float32)
        cnt = Counter(zip(genes.tolist(), lseg.tolist()))
        for (g, lp), c in cnt.items():
            j = pos[g]
            m = block_of[(j // 128, lp // 128)]
            S[j % 128, m * 128 + (lp % 128)] += c
        # kept f32 here; _prepare casts per chunk (fp8 ramp / fp16 steady)
        smats.append(S)

    chunks = _chunk_list(T2)
    mranges, t0 = [], 0
    for ch in chunks:
        m0 = sum(len(pattern[t]) for t in range(t0))
        m1 = m0 + sum(len(pattern[t]) for t in range(t0, t0 + ch))
        mranges.append((m0, m1))
        t0 += ch

    return dict(
        bounds=bounds, T2=T2, blocks=blocks,
        first_touch=first_touch, last_touch=last_touch,
        gene_pads=gene_pads, smats=smats, chunks=chunks, mranges=mranges,
    )


def _build_program(sch):
    nc = bacc.Bacc(
        "TRN2",
        target_bir_lowering=False,
        debug=False,
        num_devices=NCORES,
    )
    f16, f32, f8 = mybir.dt.float16, mybir.dt.float32, mybir.dt.float8e4

    T2 = sch["T2"]
    blocks = sch["blocks"]
    chunks = sch["chunks"]
    mranges = sch["mranges"]
    first_touch, last_touch = sch["first_touch"], sch["last_touch"]

    # Ramp chunks keep separate fp8-S + data tensors; steady chunks merge
    # x and S into ONE DRAM tensor / ONE DMA each (fewer desc issues, sem
    # lanes, and chunk-boundary waits — 20 loads measured 13.3us of Sync
    # issue time + ~60 sem ops per engine).
    t_ramp = sum(chunks[:N_FP8_CHUNKS])
    xtp_d = nc.dram_tensor("xtp", [128, t_ramp, BQ], f16, kind="ExternalInput")
    s_ds, c_ds = [], []
    for ci, (m0, m1) in enumerate(mranges):
        ch = chunks[ci]
        if ci < N_FP8_CHUNKS:
            s_ds.append(
                nc.dram_tensor(
                    f"s{ci}", [128, (m1 - m0) * 128], f8, kind="ExternalInput"
                )
            )
            c_ds.append(None)
        else:
            s_ds.append(None)
            c_ds.append(
                nc.dram_tensor(
                    f"c{ci}",
                    [128, ch * BQ + (m1 - m0) * 128],
                    f16,
                    kind="ExternalInput",
                )
            )
    inv_d = nc.dram_tensor("invsz", [128, NPT], f32, kind="ExternalInput")
    out_d = nc.dram_tensor("out", [NPT * 128, BQ], f16, kind="ExternalOutput")

    with tile.TileContext(nc) as tc:
        with (
            tc.tile_pool(name="const", bufs=1) as cpool,
            tc.tile_pool(name="warmp", bufs=1) as wpool,
            tc.tile_pool(name="psum", bufs=1, space="PSUM") as ppool,
            tc.tile_pool(name="outp", bufs=1) as opool,
        ):
            # PE warm-up: zeroed operands, separate PSUM bank. Runs while the
            # first S/x chunks stream in, flipping HAM to 8/8 early.
            warm_sb = wpool.tile([128, 640], f16, tag="warm")
            nc.gpsimd.memset(warm_sb[:], 0.0)
            warm_ps = ppool.tile([128, 512], f32, tag="wps", name="wps")
            for i in range(N_WARM):
                nc.tensor.matmul(
                    warm_ps[:],
                    warm_sb[:, 512:640],
                    warm_sb[:, 0:512],
                    start=(i == 0),
                    stop=(i == N_WARM - 1),
                )
            warm_out = wpool.tile([128, 512], f32, tag="warmo")
            nc.vector.tensor_copy(warm_out[:], warm_ps[:])

            inv_sb = cpool.tile([128, NPT], f32, tag="invsz")
            nc.scalar.dma_start(inv_sb[:], inv_d.ap())

            psb = [
                ppool.tile([128, 512], f32, tag=f"ps{n}", name=f"ps{n}")
                for n in range(NPT)
            ]
            s_sbs = []

            # Evictions (DVE) run as each bank's last matmul retires;
            # stores happen at the end via SWDGE (see below).
            ots = {}

            def evict(pt):
                ot = opool.tile([128, 512], f16, tag=f"ot{pt}", name=f"ot{pt}")
                nc.vector.tensor_scalar_mul(
                    ot[:], psb[pt][:], inv_sb[:, pt : pt + 1]
                )
                ots[pt] = ot

            t0 = 0
            last_load = None
            for ci, ch in enumerate(chunks):
                m0, m1 = mranges[ci]
                # All loads ride the Sync HWDGE ring in arrival order. (The
                # Scalar/ACT ring starves whenever the Sync ring has backlog
                # — observed 6us S-arrival delays.)
                if ci < N_FP8_CHUNKS:
                    s_sb = cpool.tile(
                        [128, (m1 - m0) * 128], f8, tag=f"s{ci}", name=f"s{ci}"
                    )
                    nc.sync.dma_start(s_sb[:], s_ds[ci].ap())
                    gt = cpool.tile(
                        [128, ch, BQ], f16, tag=f"gt{ci}", name=f"gt{ci}"
                    )
                    last_load = nc.sync.dma_start(
                        gt[:], xtp_d.ap()[:, t0 : t0 + ch, :]
                    )

                    def rhs_ap(tl, _gt=gt):
                        return _gt[:, tl, :]

                    def lhs_ap(mi, _s=s_sb):
                        return _s[:, mi * 128 : (mi + 1) * 128]
                else:
                    ct = cpool.tile(
                        [128, ch * BQ + (m1 - m0) * 128],
                        f16,
                        tag=f"c{ci}",
                        name=f"c{ci}",
                    )
                    last_load = nc.sync.dma_start(ct[:], c_ds[ci].ap())

                    def rhs_ap(tl, _ct=ct):
                        return _ct[:, tl * BQ : (tl + 1) * BQ]

                    def lhs_ap(mi, _ct=ct, _off=ch * BQ):
                        return _ct[:, _off + mi * 128 : _off + (mi + 1) * 128]

                for m in range(m0, m1):
                    tt, pt = blocks[m]
                    tl = tt - t0
                    nc.tensor.matmul(
                        psb[pt][:],
                        lhs_ap(m - m0),
                        rhs_ap(tl),
                        start=(m == first_touch[pt]),
                        stop=(m == last_touch[pt]),
                    )
                    if m == last_touch[pt]:
                        evict(pt)
                t0 += ch

            # Stores on the Scalar HWDGE ring, explicitly ordered after the
            # last load: the 8 DMAHW sem lanes are recycled in scheduled
            # order, and a store scheduled mid-stream makes a later LOAD
            # wait on store->eviction->matmul (observed 8us stall). The
            # forced dep keeps every store behind every load in lane order.
            for pt in range(NPT):
                st = nc.scalar.dma_start(
                    out_d.ap()[pt * 128 : (pt + 1) * 128, :], ots[pt][:]
                )
                bass._add_dep_helper(
                    st.ins, last_load.ins, sync=True,
                    reason="stores after all loads (DMAHW lane ordering)",
                )
    return nc


def _prepare(gene_set_features, flat_indices, segment_ids, segment_sizes):
    sch = _build_schedule(flat_indices, segment_ids)
    nc = _build_program(sch)
    nc.compile()

    x = np.asarray(gene_set_features, dtype=np.float32)
    xt16 = np.ascontiguousarray(x.T.astype(np.float16))  # (G, B)
    sizes = np.asarray(segment_sizes, dtype=np.float32)

    T2 = sch["T2"]
    bounds = sch["bounds"]
    # per-range shared arrays
    s_pieces, invs, gpads = [], [], []
    for R in range(2):
        lo_p, hi_p = bounds[R], bounds[R + 1]
        S = sch["smats"][R]
        s_pieces.append(
            [
                np.ascontiguousarray(
                    S[:, m0 * 128 : m1 * 128].astype(
                        ml_dtypes.float8_e4m3 if ci < N_FP8_CHUNKS else np.float16
                    )
                )
                for ci, (m0, m1) in enumerate(sch["mranges"])
            ]
        )
        inv = np.ones((128, NPT), np.float32)
        for pt in range(NPT):
            lo_row = lo_p + pt * 128
            n = min(128, hi_p - lo_row)
            if n > 0:
                inv[:n, pt] = 1.0 / sizes[lo_row : lo_row + n]
        invs.append(inv)
        gpads.append(
            np.array([g if g >= 0 else 0 for g in sch["gene_pads"][R]], np.int64)
        )

    chunks = sch["chunks"]
    t_ramp = sum(chunks[:N_FP8_CHUNKS])
    in_maps = []
    for c in range(NCORES):
        R, q = c // 4, c % 4
        xtp = np.ascontiguousarray(
            xt16[gpads[R], q * BQ : (q + 1) * BQ]
            .reshape(T2, 128, BQ)
            .transpose(1, 0, 2)
        )
        im = {"xtp": np.ascontiguousarray(xtp[:, :t_ramp, :]), "invsz": invs[R]}
        t0 = 0
        for ci, ch in enumerate(chunks):
            if ci < N_FP8_CHUNKS:
                im[f"s{ci}"] = s_pieces[R][ci]
            else:
                im[f"c{ci}"] = np.ascontiguousarray(
                    np.concatenate(
                        [
                            xtp[:, t0 : t0 + ch, :].reshape(128, ch * BQ),
                            s_pieces[R][ci],
                        ],
                        axis=1,
                    )
                )
            t0 += ch
        in_maps.append(im)
    return nc, in_maps, sch


def _unshard(res, sch):
    bounds = sch["bounds"]
    out = np.empty((B, P), np.float32)
    for c in range(NCORES):
        R, q = c // 4, c % 4
        lo_p, hi_p = bounds[R], bounds[R + 1]
        o = np.asarray(res.results[c]["out"]).astype(np.float32)  # (NPT*128, BQ)
        for pt in range(NPT):
            lo_row = lo_p + pt * 128
            n = min(128, hi_p - lo_row)
            if n > 0:
                out[q * BQ : (q + 1) * BQ, lo_row : lo_row + n] = o[
                    pt * 128 : pt * 128 + n
                ].T
    return np.ascontiguousarray(out)


def kernel(gene_set_features, flat_indices, segment_ids, segment_sizes, _res_hook=None):
    nc, in_maps, sch = _prepare(
        gene_set_features, flat_indices, segment_ids, segment_sizes
    )
    res = run_bass_kernel_spmd(nc, in_maps, list(range(NCORES)))
    if _res_hook is not None:
        _res_hook(res)
    return _unshard(res, sch)
